# revision 13
# baseline (speedup 1.0000x reference)
"""Multi-head attention + residual + LayerNorm on 8 Trainium2 NeuronCores.

Problem: x:[2,2048,1024] f32, 16 heads x 64 dims, full S x S softmax
attention (mask is all-ones per the input spec), out-projection, residual,
LayerNorm. Returns [2,2048,1024] f32.

Sharding: tensor-parallel over heads for QKV+attention (2 heads/core), then an
AllToAll that redistributes the normalized per-head context from
head-sharded [128 dims, 4096 rows] to row-sharded [1024 dims, 512 rows],
after which each core computes the output projection + residual + LayerNorm
for its own 512 rows of the flattened (B*S, D) activation.

Compute dtype fp8e4m3 on the TensorEngine (fp32 PSUM accumulation), using
MatmulPerfMode.DoubleRow (paired K-planes, 2x rate) for every contraction
with K>=256: QKV projections, attn@V, and the out-projection. Scores (K=64)
run as plain fp8 matmuls.

Scheduling: the attend loop is software-pipelined with depth-2 deferral --
the ctx matmul for score-group u is emitted two groups later, so the PE
FIFO never blocks on an exp that hasn't finished, keeping ACT/DVE (the
bottleneck engines, which carry the softmax exp) continuously fed. The
q/k/v projections for the *other* batch are emitted as fine-grained pieces
woven one-per-group into the attend stream, filling the PE slack instead
of serializing after attention. The per-chunk softmax epilogue (bit-trick
reciprocal of the denominator row, broadcast via a tiny matmul, fp8
normalize) is emitted in the one-group window between a chunk's last ctx
matmul and the next chunk's first.

fp8 scaling: weights have std 0.02 which sits below fp8e4m3's minimum
normal (2^-6); Wq/Wk are stored x16 (descaled in the softmax exp scale)
and Wo x32 (descaled, with the x1024 reciprocal scale, by 2^-15 fused
into the residual add).

Softmax skips the max-subtraction (scores are O(1)), gets the denominator
free via a ones column appended to the V stationary operand, and splits the
exp work over ACT (native Exp, fp8 out) and DVE (Schraudolph bit-trick exp:
one fused multiply-add with f32->int8 cast; the int8 bit pattern read back
as fp8e4m3 is 2^(x/ln2 * 8)/2^56 ~ e^x to ~4%). The denominator row sums
the same quantized values the ctx matmul consumes, so softmax stays exactly
normalized and the approximation error is only a few-percent reweighting
noise.

LayerNorm rstd uses ACT Sqrt + DVE reciprocal, with the sqrt table set
preloaded (on idle ACT, behind the AllToAll) so the exp->sqrt table
switch never sits on the critical path.

All-ones mask and zero/nonzero biases are handled exactly; a non-trivial
mask (impossible per the input spec, which pins fill=ones) falls back to a
numpy reference path.
"""

import sys

sys.path.insert(0, "/opt/trn_rl_repo")

from collections import deque

import numpy as np
import ml_dtypes

import concourse.bass as bass
import concourse.bacc as bacc
import concourse.mybir as mybir
import concourse.tile as tile
from concourse.bass_utils import run_bass_kernel_spmd

B, S, D, H = 2, 2048, 1024, 16
HD = D // H  # 64
NORM = 1.0 / float(np.sqrt(HD))
EPS = 1e-5
NC = 8  # cores
HLOC = H // NC  # 2 heads per core
ROWS = B * S  # 4096 flattened rows
RLOC = ROWS // NC  # 512 rows per core
KT = S // 128  # 16 k-tiles per batch
QC = S // 512  # 4 q-chunks of 512 per batch

# fp8 scale plan (see module docstring). fp8e4m3 here is the IEEE variant
# (max finite 240, then inf), so keep every fp8 tensor's 6-sigma under 240.
SQ = 16.0  # Wq scale -> q8 std ~10
SK = 16.0  # Wk scale
SV = 1.0  # Wv scale -> v8 std ~0.64, ctxN std ~19
SW = 32.0  # Wo scale -> wo8 std ~0.64
SR = 1024.0  # reciprocal scale: rec8 = SR/denom ~ 0.5
OUT_DESCALE = 1.0 / (SV * SR * SW)  # 2^-18, applied at the residual add
EXP_SCALE = NORM / (SQ * SK)  # descales q8.k8 scores inside exp
# Schraudolph exp to fp8e4m3 bits: i8 = round-ish(x * 8/ln2 + 56); the
# +0.22 offset centers the truncating f32->int cast's downward bias.
SCH_A = 8.0 / float(np.log(2.0))
SCH_B = 55.72
# bit-trick reciprocal: bits(1/x) ~ K - bits(x), max rel err ~4%, which the
# softmax tolerates (it scales each row's weights uniformly).
RCP_K = 0x7EF0A3D7

f32 = mybir.dt.float32
bf16 = mybir.dt.bfloat16
f8 = mybir.dt.float8e4
i8 = mybir.dt.int8
i32 = mybir.dt.int32
AF = mybir.ActivationFunctionType
OP = mybir.AluOpType
DR = mybir.MatmulPerfMode.DoubleRow

_CACHE = {}


def _build(sim1=False, reps=1, ln_affine=False, zb=True, ablate=None):
    nc = bacc.Bacc(trn_type="TRN2", num_devices=1 if sim1 else NC)

    # weights host-prearranged to DoubleRow pair layout [p, j, i, m]:
    # contraction dim = 128*(2j+i) + p
    xT8_d = nc.declare_dram_parameter("xT8", [128, 4, 2, ROWS], f8, isOutput=False)
    xb_d = nc.declare_dram_parameter("xb", [RLOC, D], f32, isOutput=False)
    wq_d = nc.declare_dram_parameter("wq", [128, 4, 2, 128], f8, isOutput=False)
    wk_d = nc.declare_dram_parameter("wk", [128, 4, 2, 128], f8, isOutput=False)
    wv_d = nc.declare_dram_parameter("wv", [128, 4, 2, 128], f8, isOutput=False)
    wo_d = nc.declare_dram_parameter("wo", [128, 4, 2, D], f8, isOutput=False)
    if not zb:
        bq_d = nc.declare_dram_parameter("bq", [128, 1], f32, isOutput=False)
        bk_d = nc.declare_dram_parameter("bk", [128, 1], f32, isOutput=False)
        bv_d = nc.declare_dram_parameter("bv", [64, HLOC], f32, isOutput=False)
    if ln_affine:
        gam_d = nc.declare_dram_parameter("gamma", [D], f32, isOutput=False)
        bet_d = nc.declare_dram_parameter("beta", [D], f32, isOutput=False)
    out_d = nc.declare_dram_parameter("out", [RLOC, D], f32, isOutput=True)

    with tile.TileContext(nc) as tc:
        with (
            tc.tile_pool(name="singles", bufs=1) as singles,
            tc.tile_pool(name="temps", bufs=4) as temps,
            tc.tile_pool(name="psA", bufs=2, space="PSUM") as psA,
            tc.tile_pool(name="psB", bufs=2, space="PSUM") as psB,
            tc.tile_pool(name="psP", bufs=2, space="PSUM") as psP,
            tc.tile_pool(name="dram", bufs=1, space="DRAM") as dram,
        ):
            for _rep in range(reps):  # >1 only for benchmarking
                # AllToAll buffers, fp8 normalized ctx^T. Input rows
                # [128*o : 128*(o+1)] hold this core's 128 head-dims for
                # owner-core o's 512 q-rows; output rows [128*r : ...] are
                # core r's head dims (= global dims 128r..) for MY rows.
                a2a_in = dram.tile([NC * 128, RLOC], f8)
                a2a_out = dram.tile([NC * 128, RLOC], f8)

                # ---- small constants first ----
                wq_sb = singles.tile([128, 4, 2, 128], f8)
                wk_sb = singles.tile([128, 4, 2, 128], f8)
                wv_sb = singles.tile([128, 4, 2, 128], f8)
                for w_sb, w_d in ((wq_sb, wq_d), (wk_sb, wk_d), (wv_sb, wv_d)):
                    nc.sync.dma_start(w_sb, w_d[:, :, :, :])
                if not zb:
                    bq_sb = singles.tile([128, 1], f32)
                    nc.sync.dma_start(bq_sb, bq_d[:, :])
                    bk_sb = singles.tile([128, 1], f32)
                    nc.sync.dma_start(bk_sb, bk_d[:, :])
                    bv_sb = singles.tile([64, HLOC], f32)
                    nc.sync.dma_start(bv_sb, bv_d[:, :])
                ones_sb = singles.tile([1, 64], f8)
                nc.vector.memset(ones_sb, 1.0)
                eps_sb = singles.tile([128, 1], f32)
                nc.vector.memset(eps_sb, EPS)
                rsq_dummy = singles.tile([1, 1], f32)

                # ---- x^T fp8 pair-tiles: 8 stripes of 512KB (DMA calls cost
                # ~2.2us each on HW, so few big transfers), batch-0 columns
                # first, split across the SP and ACT hardware DGE queues ----
                xT_sb = [
                    singles.tile([128, 2, ROWS], f8, tag=f"xT{j}", name=f"xT{j}")
                    for j in range(4)
                ]
                for b in range(B):
                    for j in range(4):
                        eng = nc.sync if j % 2 == 0 else nc.scalar
                        eng.dma_start(
                            xT_sb[j][:, :, b * S : (b + 1) * S],
                            xT8_d[:, j, :, b * S : (b + 1) * S],
                        )

                # persistent per-batch projection outputs (separate tiles so
                # batch-1 writes don't false-WAR batch-0 attention reads)
                qT_sb = [
                    singles.tile([128, S], f8, tag=f"qT{b}", name=f"qT{b}")
                    for b in range(B)
                ]
                kT_sb = [
                    singles.tile([128, S], f8, tag=f"kT{b}", name=f"kT{b}")
                    for b in range(B)
                ]
                # v stationary per (head, ktile), padded to 128 columns for
                # DoubleRow ldweights (M must be 64-contiguous or 128):
                # cols 0-63 = v, col 64 = ones (denominator row), 65-127
                # unused (zeroed once; their PSUM rows are never read)
                v_sb = [
                    singles.tile([128, HLOC, KT, 128], f8, tag=f"v{b}", name=f"v{b}")
                    for b in range(B)
                ]
                nc.vector.memset(v_sb[0], 0.0)
                nc.gpsimd.memset(v_sb[1], 0.0)
                for b in range(B):
                    nc.vector.memset(v_sb[b][:, :, :, 64:65], 1.0)

                def qk_copy(dst, src, bias_sb, eng):
                    """PSUM f32 -> SBUF fp8 (+bias when not zb).

                    GPSIMD can't read PSUM, so only ACT ("A") / DVE ("D")."""
                    if zb:
                        if eng == "A":
                            nc.scalar.activation(out=dst, in_=src, func=AF.Copy)
                        else:
                            nc.vector.tensor_copy(dst, src)
                    else:
                        if eng == "A":
                            nc.scalar.activation(
                                out=dst, in_=src, func=AF.Identity, bias=bias_sb
                            )
                        else:
                            nc.vector.tensor_scalar_add(dst, src, bias_sb)

                def qk_piece(b, sc, which, eng):
                    """Generator: one q or k projection chunk, yielding after
                    each DR matmul and after the copy-out."""
                    lo = b * S + sc * 512
                    w_sb = wq_sb if which == "q" else wk_sb
                    dstT = qT_sb[b] if which == "q" else kT_sb[b]
                    bias = None
                    if not zb:
                        bias = bq_sb if which == "q" else bk_sb
                    ps = psP.tile([128, 512], f32, tag="p", name=f"ps{which}{b}{sc}")
                    for j in range(4):
                        nc.tensor.matmul(
                            ps,
                            w_sb[:, j],
                            xT_sb[j][:, :, lo : lo + 512],
                            start=(j == 0),
                            stop=(j == 3),
                            perf_mode=DR,
                        )
                        yield
                    qk_copy(dstT[:, sc * 512 : (sc + 1) * 512], ps, bias, eng)
                    yield

                def v_piece(b, rt, eng):
                    """Generator: one v projection row-tile (x-rows stationary
                    so the PSUM comes out [x-rows, v-dims], matching the
                    attn@V stationary layout with no transpose)."""
                    lo = b * S + rt * 128
                    psv = psP.tile([128, 128], f32, tag="p", name=f"psv{b}{rt}")
                    for j in range(4):
                        nc.tensor.matmul(
                            psv,
                            xT_sb[j][:, :, lo : lo + 128],
                            wv_sb[:, j],
                            start=(j == 0),
                            stop=(j == 3),
                            perf_mode=DR,
                        )
                        if j % 2 == 1:
                            yield
                    dst = v_sb[b][:, :, rt, 0:64]
                    src = psv.rearrange("p (h c) -> p h c", c=64)
                    if eng == "A":
                        nc.scalar.activation(out=dst, in_=src, func=AF.Copy)
                    else:
                        nc.vector.tensor_copy(dst, src)
                    yield

                def chain(*gens):
                    for g in gens:
                        yield from g

                # exp engines: ACT (native Exp) and DVE (schraudolph); GPSIMD
                # can't read the PSUM scores.
                def emit_exp(eng, ex, sg):
                    flat = ex.rearrange("p i n -> p (i n)")
                    if eng == "A":
                        nc.scalar.activation(
                            out=flat, in_=sg, func=AF.Exp, scale=EXP_SCALE
                        )
                    else:
                        nc.vector.tensor_scalar(
                            out=flat.bitcast(i8),
                            in0=sg,
                            scalar1=EXP_SCALE * SCH_A,
                            scalar2=SCH_B,
                            op0=OP.mult,
                            op1=OP.add,
                        )

                # per-head accumulation buffers for the normalized ctx of all
                # four q-chunks of a batch; shipped as ONE DMA per (b, h)
                ctxN_all = [
                    singles.tile([64, QC, 512], f8, tag=f"cNA{h}", name=f"cNA{h}")
                    for h in range(HLOC)
                ]

                def attend_epilogue(b, qc, ctx_ps):
                    """normalize both heads' ctx^T by their denominator rows
                    (bit-trick reciprocal), cast fp8, stage into ctxN_all.
                    DVE ops may read at most one PSUM operand, so the
                    broadcast reciprocal plane goes PSUM -> SBUF once."""
                    rep = psP.tile([128, 512], f32, tag="p", name="rep")
                    for h in range(HLOC):
                        rec = temps.tile([1, 512], f32, tag="rec", name=f"rec{h}")
                        nc.vector.tensor_scalar(
                            out=rec.bitcast(i32),
                            in0=ctx_ps[h][64:65, :].bitcast(i32),
                            scalar1=-1,
                            scalar2=RCP_K,
                            op0=OP.mult,
                            op1=OP.add,
                        )
                        rec8 = temps.tile([1, 512], f8, tag="rec8", name=f"rec8{h}")
                        nc.gpsimd.tensor_scalar_mul(rec8, rec, float(SR))
                        nc.tensor.matmul(
                            rep[h * 64 : (h + 1) * 64, :],
                            ones_sb,
                            rec8,
                            start=True,
                            stop=True,
                            tile_position=(0, h * 64),
                        )
                    repS = temps.tile([128, 512], f32, tag="repS")
                    nc.vector.tensor_copy(repS, rep)
                    for h in range(HLOC):
                        hp = h * 64
                        ctxN = ctxN_all[h][:, qc, :]
                        if zb:
                            nc.vector.tensor_mul(
                                ctxN, ctx_ps[h][0:64, :], repS[hp : hp + 64, :]
                            )
                        else:
                            ctxE = temps.tile([64, 512], f32, tag="ctxE")
                            nc.vector.tensor_mul(
                                ctxE, ctx_ps[h][0:64, :], repS[hp : hp + 64, :]
                            )
                            # bv_sb arrives host-scaled by SV*SR
                            nc.vector.tensor_scalar_add(
                                ctxN, ctxE, bv_sb[:, h : h + 1]
                            )

                def ship_batch(b):
                    """one strided DMA per head: [64, QC, 512] -> a2a_in rows
                    {(b*QC+qc)*128 + h*64 + d}."""
                    a2a_v = a2a_in.rearrange("(o p) n -> p o n", p=128)
                    for h in range(HLOC):
                        hp = h * 64
                        nc.sync.dma_start(
                            a2a_v[hp : hp + 64, b * QC : (b + 1) * QC, :],
                            ctxN_all[h],
                        )

                ng = KT // 4 if ablate == "half_attend" else KT // 2

                def attend_batch(b, weave=None, aux=(), dve_mod=(2, 3), rate=1):
                    """scores (fp8, K=64) -> exp (ACT/DVE, fp8 out) -> ctx^T
                    via DoubleRow over k-tile pairs (+denom row), with depth-2
                    deferral of the ctx matmuls and the normalize epilogue
                    emitted in the window between chunks.

                    weave: generator whose next() emits one foreign PE piece;
                    `rate` pieces are consumed per unit.
                    aux: callables emitted one per unit from unit 0 (used to
                    flush the previous batch's deferred tail).
                    dve_mod: (2g+h) % 8 values routed to DVE exp.
                    Returns (pend, last_epi) for the caller to flush.
                    """
                    aux = deque(aux)
                    pend = deque()  # (ctx_ps, h, ex, g)
                    last_epi = None
                    done = object()  # generators yield None; sentinel differs

                    def flush_one():
                        ctx_ps_, h_, ex_, g_ = pend.popleft()
                        nc.tensor.matmul(
                            ctx_ps_[h_],
                            v_sb[b][:, h_, 2 * g_ : 2 * g_ + 2, :],
                            ex_[:, :, :],
                            start=(g_ == 0),
                            stop=(g_ == ng - 1),
                            perf_mode=DR,
                        )

                    for qc in range(QC):
                        qlo = qc * 512
                        ctx_ps = [
                            psB.tile([128, 512], f32, tag="b", name=f"ctx{h}")
                            for h in range(HLOC)
                        ]
                        for g in range(ng):
                            for h in range(HLOC):
                                hp = h * 64
                                sg = psA.tile(
                                    [128, 1024], f32, tag="a", name=f"sg{h}"
                                )
                                for jj in range(2):
                                    klo = (2 * g + jj) * 128
                                    nc.tensor.matmul(
                                        sg[:, jj * 512 : (jj + 1) * 512],
                                        kT_sb[b][hp : hp + 64, klo : klo + 128],
                                        qT_sb[b][hp : hp + 64, qlo : qlo + 512],
                                        start=True,
                                        stop=True,
                                    )
                                ex = temps.tile(
                                    [128, 2, 512], f8, tag="exps", name=f"ex{h}"
                                )
                                eng = "D" if (2 * g + h) % 8 in dve_mod else "A"
                                emit_exp(eng, ex, sg)
                                if aux:
                                    aux.popleft()()
                                for _ in range(rate):
                                    if weave is None:
                                        break
                                    if next(weave, done) is done:
                                        weave = None
                                pend.append((ctx_ps, h, ex, g))
                                if len(pend) > 2:
                                    flush_one()
                                # window between chunks: the previous chunk's
                                # last ctx was just flushed above; normalize
                                # it before the next flush (the new chunk's
                                # first ctx) reuses its PSUM slot
                                if g == 0 and h == 1 and last_epi is not None:
                                    attend_epilogue(*last_epi)
                                    last_epi = None
                        last_epi = (b, qc, ctx_ps)
                    return pend, last_epi

                def flush_attend(b, pend, last_epi):
                    while pend:
                        ctx_ps_, h_, ex_, g_ = pend.popleft()
                        nc.tensor.matmul(
                            ctx_ps_[h_],
                            v_sb[b][:, h_, 2 * g_ : 2 * g_ + 2, :],
                            ex_[:, :, :],
                            start=(g_ == 0),
                            stop=(g_ == ng - 1),
                            perf_mode=DR,
                        )
                    if last_epi is not None:
                        attend_epilogue(*last_epi)

                # ---- batch-0 projection head: k first, then q chunk 0, then
                # all of v (batch 0's first q-chunk consumes every v k-tile
                # within its first 16 units, so v0 can't be woven); q chunks
                # 1-3 and all of batch 1's projections weave into the attend
                # streams as PE-slack filler ----
                for sc in range(QC):
                    for _ in qk_piece(0, sc, "k", "A"):
                        pass
                for _ in qk_piece(0, 0, "q", "A"):
                    pass
                for rt in range(KT):
                    for _ in v_piece(0, rt, "A"):
                        pass

                # 64 yields, one per batch-0 attend unit; ordered so each
                # piece lands before its first reader (q0 chunk sc is read
                # from unit 16*sc; batch-1 pieces are read in attend 1)
                weave0 = chain(
                    qk_piece(0, 1, "q", "D"),
                    qk_piece(0, 2, "q", "D"),
                    qk_piece(0, 3, "q", "D"),
                    *[qk_piece(1, sc, "k", "D") for sc in range(QC)],
                    qk_piece(1, 0, "q", "D"),
                    *[v_piece(1, rt, "D") for rt in range(0, 8)],
                )
                pend0, epi0 = attend_batch(0, weave=weave0, dve_mod=(2, 3))

                # batch-1 leftovers + phase-4 constant loads weave into the
                # batch-1 attend stream (their deadlines are chunks 1-3)
                def load_tail_consts():
                    wo_sb_l = singles.tile([128, 4, 2, D], f8, name="wo_sb")
                    nc.sync.dma_start(wo_sb_l, wo_d[:, :, :, :])
                    tail_tiles["wo"] = wo_sb_l
                    if ln_affine:
                        gam_sb = singles.tile([128, D], f32, name="gam_sb")
                        gap = gam_d.ap()
                        nc.sync.dma_start(
                            gam_sb,
                            bass.AP(
                                tensor=gap.tensor,
                                offset=gap.offset,
                                ap=[[0, 128], gap.ap[0]],
                            ),
                        )
                        tail_tiles["gam"] = gam_sb
                        bet_sb = singles.tile([128, D], f32, name="bet_sb")
                        bap = bet_d.ap()
                        nc.sync.dma_start(
                            bet_sb,
                            bass.AP(
                                tensor=bap.tensor,
                                offset=bap.offset,
                                ap=[[0, 128], bap.ap[0]],
                            ),
                        )
                        tail_tiles["bet"] = bet_sb
                    xb_sb_l = singles.tile([128, RLOC // 128, D], f32, name="xb_sb")
                    nc.sync.dma_start(
                        xb_sb_l, xb_d.ap().rearrange("(t p) d -> p t d", p=128)
                    )
                    tail_tiles["xb"] = xb_sb_l

                tail_tiles = {}
                # batch-1 leftovers: v k-tiles 8-15 are read from unit 10 on
                # (ctx for group g lands at unit 2g+3), so consume 2 pieces
                # per unit until the generator drains (~unit 19)
                weave1 = chain(
                    *[v_piece(1, rt, "D") for rt in range(8, 12)],
                    qk_piece(1, 1, "q", "D"),
                    *[v_piece(1, rt, "D") for rt in range(12, KT)],
                    qk_piece(1, 2, "q", "D"),
                    qk_piece(1, 3, "q", "D"),
                )

                def flush0_ctx():
                    # flush batch 0's two deferred ctx matmuls
                    while pend0:
                        ctx_ps_, h_, ex_, g_ = pend0.popleft()
                        nc.tensor.matmul(
                            ctx_ps_[h_],
                            v_sb[0][:, h_, 2 * g_ : 2 * g_ + 2, :],
                            ex_[:, :, :],
                            start=(g_ == 0),
                            stop=(g_ == ng - 1),
                            perf_mode=DR,
                        )

                def epi0_fn():
                    attend_epilogue(*epi0)

                def ship0_fn():
                    ship_batch(0)
                    load_tail_consts()

                pend1, epi1 = attend_batch(
                    1,
                    weave=weave1,
                    aux=(flush0_ctx, epi0_fn, ship0_fn),
                    dve_mod=(2, 3, 4),
                    rate=2,
                )
                flush_attend(1, pend1, epi1)
                # preload the Sqrt table set while the AllToAll runs (ACT is
                # idle; keeps the exp->sqrt switch off the critical path)
                nc.scalar.activation(
                    out=rsq_dummy, in_=eps_sb[0:1, :], func=AF.Sqrt
                )
                ship_batch(1)

                # ---- AllToAll: head-sharded ctx^T -> row-sharded ctx^T ----
                if ablate == "no_a2a":
                    a2a_out = a2a_in
                elif sim1 or ablate == "local_a2a":
                    for o in range(NC):
                        nc.sync.dma_start(
                            a2a_out[o * 128 : (o + 1) * 128, :],
                            a2a_in[o * 128 : (o + 1) * 128, :],
                        )
                else:
                    nc.gpsimd.collective_compute(
                        "AllToAll",
                        OP.bypass,
                        replica_groups=[list(range(NC))],
                        ins=[a2a_in.opt()],
                        outs=[a2a_out.opt()],
                    )

                # ---- out-projection (DoubleRow) + residual + LayerNorm,
                # software-pipelined: pso is one [128,1024] tile per row-tile
                # (2 in flight), so the next row-tile's matmuls never wait on
                # this one's DVE epilogue ----
                wo_sb = tail_tiles["wo"]
                xb_sb = tail_tiles["xb"]
                ct_sb = [
                    singles.tile([128, 2, RLOC], f8, tag=f"ct{j}", name=f"ct{j}")
                    for j in range(4)
                ]
                a2a_ov = a2a_out.rearrange("(j i p) n -> p j i n", i=2, p=128)
                for j in range(4):
                    eng = nc.sync if j % 2 == 0 else nc.scalar
                    eng.dma_start(ct_sb[j], a2a_ov[:, j, :, :])

                def tail_mms(t):
                    pso = psA.tile([128, D], f32, tag="a", name=f"pso{t}")
                    for eh in range(2):
                        for j in range(4):
                            nc.tensor.matmul(
                                pso[:, eh * 512 : (eh + 1) * 512],
                                ct_sb[j][:, :, t * 128 : (t + 1) * 128],
                                wo_sb[:, j, :, eh * 512 : (eh + 1) * 512],
                                start=(j == 0),
                                stop=(j == 3),
                                perf_mode=DR,
                            )
                    return pso

                def tail_epilogue(t, pso):
                    y_sb = temps.tile([128, D], f32, tag="y")
                    # residual with fp8 descale: y = out*2^-18 + (x + bo)
                    # (scalar_tensor_tensor is DVE-only: walrus rejects it on
                    # the Pool engine)
                    for eh in range(2):
                        nc.vector.scalar_tensor_tensor(
                            out=y_sb[:, eh * 512 : (eh + 1) * 512],
                            in0=pso[:, eh * 512 : (eh + 1) * 512],
                            scalar=OUT_DESCALE,
                            in1=xb_sb[:, t, eh * 512 : (eh + 1) * 512],
                            op0=OP.mult,
                            op1=OP.add,
                        )
                    # LayerNorm over D=1024 (free dim)
                    stats = temps.tile([128, 2, 6], f32, tag="stats")
                    for i in range(2):
                        nc.vector.bn_stats(
                            out=stats[:, i, :],
                            in_=y_sb[:, i * 512 : (i + 1) * 512],
                        )
                    mv = temps.tile([128, 2], f32, tag="mv")
                    nc.vector.bn_aggr(out=mv, in_=stats)
                    # rstd = 1/Sqrt(var + eps); table preloaded behind the a2a
                    nc.scalar.activation(
                        out=mv[:, 1:2], in_=mv[:, 1:2], func=AF.Sqrt, bias=eps_sb
                    )
                    nc.vector.reciprocal(mv[:, 1:2], mv[:, 1:2])
                    # y_norm = (y - mean) * rstd on GPSIMD (SBUF-only engine)
                    yo = temps.tile([128, D], f32, tag="yo")
                    nc.gpsimd.tensor_scalar(
                        out=yo,
                        in0=y_sb,
                        scalar1=mv[:, 0:1],
                        scalar2=mv[:, 1:2],
                        op0=OP.subtract,
                        op1=OP.mult,
                    )
                    if ln_affine:
                        nc.vector.tensor_mul(yo, yo, tail_tiles["gam"])
                        nc.gpsimd.tensor_add(yo, yo, tail_tiles["bet"])
                    nc.sync.dma_start(out_d[t * 128 : (t + 1) * 128, :], yo)

                nt = 0 if ablate == "no_tail" else RLOC // 128
                prev = None
                for t in range(nt):
                    pso = tail_mms(t)
                    if prev is not None:
                        tail_epilogue(*prev)
                    prev = (t, pso)
                if prev is not None:
                    tail_epilogue(*prev)

    nc.compile()
    return nc


def _numpy_reference(x, mask, Wq, bq, Wk, bk, Wv, bv, Wo, bo, gamma, beta):
    """Fallback for a non-all-ones mask (can't occur per the input spec)."""
    b = x.shape[0]
    x64 = x.astype(np.float64)

    def split(t):
        return t.reshape(b, -1, H, HD).transpose(0, 2, 1, 3)

    q = split(x64 @ Wq + bq)
    k = split(x64 @ Wk + bk)
    v = split(x64 @ Wv + bv)
    scores = np.einsum("bhqd,bhkd->bhqk", q, k) * NORM
    scores = np.where(mask == 0, -1e9, scores)
    scores -= scores.max(axis=-1, keepdims=True)
    e = np.exp(scores)
    attn = e / e.sum(axis=-1, keepdims=True)
    ctx = np.einsum("bhqk,bhkd->bhqd", attn, v)
    ctx = ctx.transpose(0, 2, 1, 3).reshape(b, -1, D)
    out = ctx @ Wo + bo
    y = out + x64
    mu = y.mean(-1, keepdims=True)
    var = y.var(-1, keepdims=True)
    return ((y - mu) / np.sqrt(var + EPS) * gamma + beta).astype(np.float32)


def kernel(x, mask, Wq, bq, Wk, bk, Wv, bv, Wo, bo, gamma, beta):
    x = np.asarray(x, dtype=np.float32)
    mask = np.asarray(mask)
    Wq, bq = np.asarray(Wq, np.float32), np.asarray(bq, np.float32)
    Wk, bk = np.asarray(Wk, np.float32), np.asarray(bk, np.float32)
    Wv, bv = np.asarray(Wv, np.float32), np.asarray(bv, np.float32)
    Wo, bo = np.asarray(Wo, np.float32), np.asarray(bo, np.float32)
    gamma, beta = np.asarray(gamma, np.float32), np.asarray(beta, np.float32)

    if not np.all(mask):
        return _numpy_reference(x, mask, Wq, bq, Wk, bk, Wv, bv, Wo, bo, gamma, beta)

    ln_affine = not (np.all(gamma == 1.0) and np.all(beta == 0.0))
    zb = bool(
        np.all(bq == 0.0) and np.all(bk == 0.0) and np.all(bv == 0.0)
    )
    key = ("nc", ln_affine, zb)
    if key not in _CACHE:
        _CACHE[key] = _build(ln_affine=ln_affine, zb=zb)
    nc = _CACHE[key]

    e4 = ml_dtypes.float8_e4m3

    def to_pair(w, scale):  # [1024, M] -> [128, 4, 2, M], contraction 128*(2j+i)+p
        m = w.shape[1]
        return np.ascontiguousarray(
            (w * scale).reshape(4, 2, 128, m).transpose(2, 0, 1, 3)
        ).astype(e4)

    x2 = x.reshape(ROWS, D)
    xT8 = np.ascontiguousarray(
        x2.T.reshape(4, 2, 128, ROWS).transpose(2, 0, 1, 3)
    ).astype(e4)
    wo_8 = to_pair(Wo, SW)
    in_maps = []
    for c in range(NC):
        hc = c * HLOC  # first head on this core
        d0 = hc * HD  # its first column/row in the D dim
        m = {
            "xT8": xT8,
            "xb": np.ascontiguousarray(x2[c * RLOC : (c + 1) * RLOC]) + bo,
            "wq": to_pair(Wq[:, d0 : d0 + 128], SQ),
            "wk": to_pair(Wk[:, d0 : d0 + 128], SK),
            "wv": to_pair(Wv[:, d0 : d0 + 128], SV),
            "wo": wo_8,
        }
        if not zb:
            m["bq"] = (np.ascontiguousarray(bq[d0 : d0 + 128]) * SQ).reshape(128, 1)
            m["bk"] = (np.ascontiguousarray(bk[d0 : d0 + 128]) * SK).reshape(128, 1)
            m["bv"] = np.ascontiguousarray(
                bv[d0 : d0 + 128].reshape(HLOC, HD).T * (SV * SR)
            )
        if ln_affine:
            m["gamma"] = gamma
            m["beta"] = beta
        in_maps.append(m)

    res = run_bass_kernel_spmd(nc, in_maps, list(range(NC)))
    out = np.concatenate([res.results[c]["out"] for c in range(NC)], axis=0)
    return out.reshape(B, S, D).astype(np.float32)


# revision 16
# speedup vs baseline: 1.1916x; 1.1916x over previous
"""Multi-head attention + residual + LayerNorm on 8 Trainium2 NeuronCores.

Problem: x:[2,2048,1024] f32, 16 heads x 64 dims, full S x S softmax
attention (mask is all-ones per the input spec), out-projection, residual,
LayerNorm. Returns [2,2048,1024] f32.

Sharding: tensor-parallel over heads for QKV+attention (2 heads/core), then an
AllToAll that redistributes the normalized per-head context from
head-sharded [128 dims, 4096 rows] to row-sharded [1024 dims, 512 rows],
after which each core computes the output projection + residual + LayerNorm
for its own 512 rows of the flattened (B*S, D) activation.

Compute dtype fp8e4m3 on the TensorEngine (fp32 PSUM accumulation), using
MatmulPerfMode.DoubleRow (paired K-planes, 2x rate) for every contraction
with K>=256: QKV projections, attn@V, and the out-projection. Scores (K=64)
run as plain fp8 matmuls.

Scheduling: the attend loop is software-pipelined with depth-2 deferral --
the ctx matmul for score-group u is emitted two groups later, so the PE
FIFO never blocks on an exp that hasn't finished, keeping ACT/DVE (the
bottleneck engines, which carry the softmax exp) continuously fed. The
q/k/v projections for the *other* batch are emitted as fine-grained pieces
woven one-per-group into the attend stream, filling the PE slack instead
of serializing after attention. The per-chunk softmax epilogue (bit-trick
reciprocal of the denominator row, broadcast via a tiny matmul, fp8
normalize) is emitted in the one-group window between a chunk's last ctx
matmul and the next chunk's first.

fp8 scaling: weights have std 0.02 which sits below fp8e4m3's minimum
normal (2^-6); Wq/Wk are stored x16 (descaled in the softmax exp scale)
and Wo x32 (descaled, with the x1024 reciprocal scale, by 2^-15 fused
into the residual add).

Softmax skips the max-subtraction (scores are O(1)), gets the denominator
free via a ones column appended to the V stationary operand, and splits the
exp work over ACT (native Exp, fp8 out) and DVE (Schraudolph bit-trick exp:
one fused multiply-add with f32->int8 cast; the int8 bit pattern read back
as fp8e4m3 is 2^(x/ln2 * 8)/2^56 ~ e^x to ~4%). The denominator row sums
the same quantized values the ctx matmul consumes, so softmax stays exactly
normalized and the approximation error is only a few-percent reweighting
noise.

LayerNorm rstd uses ACT Sqrt + DVE reciprocal, with the sqrt table set
preloaded (on idle ACT, behind the AllToAll) so the exp->sqrt table
switch never sits on the critical path.

All-ones mask and zero/nonzero biases are handled exactly; a non-trivial
mask (impossible per the input spec, which pins fill=ones) falls back to a
numpy reference path.
"""

import sys

sys.path.insert(0, "/opt/trn_rl_repo")

from collections import deque

import numpy as np
import ml_dtypes

import concourse.bass as bass
import concourse.bacc as bacc
import concourse.mybir as mybir
import concourse.tile as tile
from concourse.bass_utils import run_bass_kernel_spmd

B, S, D, H = 2, 2048, 1024, 16
HD = D // H  # 64
NORM = 1.0 / float(np.sqrt(HD))
EPS = 1e-5
NC = 8  # cores
HLOC = H // NC  # 2 heads per core
ROWS = B * S  # 4096 flattened rows
RLOC = ROWS // NC  # 512 rows per core
KT = S // 128  # 16 k-tiles per batch
QC = S // 512  # 4 q-chunks of 512 per batch

# fp8 scale plan (see module docstring). fp8e4m3 here is the IEEE variant
# (max finite 240, then inf), so keep every fp8 tensor's 6-sigma under 240.
SQ = 16.0  # Wq scale -> q8 std ~10
SK = 16.0  # Wk scale
SV = 1.0  # Wv scale -> v8 std ~0.64, ctxN std ~19
SW = 32.0  # Wo scale -> wo8 std ~0.64
SR = 1024.0  # reciprocal scale: rec8 = SR/denom ~ 0.5
OUT_DESCALE = 1.0 / (SV * SR * SW)  # 2^-18, applied at the residual add
EXP_SCALE = NORM / (SQ * SK)  # descales q8.k8 scores inside exp
# Schraudolph exp to fp8e4m3 bits: i8 = round-ish(x * 8/ln2 + 56); the
# +0.22 offset centers the truncating f32->int cast's downward bias.
SCH_A = 8.0 / float(np.log(2.0))
SCH_B = 55.72
# bit-trick reciprocal: bits(1/x) ~ K - bits(x), max rel err ~4%, which the
# softmax tolerates (it scales each row's weights uniformly).
RCP_K = 0x7EF0A3D7

f32 = mybir.dt.float32
bf16 = mybir.dt.bfloat16
f8 = mybir.dt.float8e4
i8 = mybir.dt.int8
i32 = mybir.dt.int32
AF = mybir.ActivationFunctionType
OP = mybir.AluOpType
DR = mybir.MatmulPerfMode.DoubleRow

_CACHE = {}


def _build(sim1=False, reps=1, ln_affine=False, zb=True, ablate=None, WEAVE=True):
    nc = bacc.Bacc(trn_type="TRN2", num_devices=1 if sim1 else NC)

    # weights host-prearranged to DoubleRow pair layout [p, j, i, m]:
    # contraction dim = 128*(2j+i) + p
    xT8_d = nc.declare_dram_parameter("xT8", [128, 4, 2, ROWS], f8, isOutput=False)
    xb_d = nc.declare_dram_parameter("xb", [RLOC, D], f32, isOutput=False)
    wq_d = nc.declare_dram_parameter("wq", [128, 4, 2, 128], f8, isOutput=False)
    wk_d = nc.declare_dram_parameter("wk", [128, 4, 2, 128], f8, isOutput=False)
    wv_d = nc.declare_dram_parameter("wv", [128, 4, 2, 128], f8, isOutput=False)
    wo_d = nc.declare_dram_parameter("wo", [128, 4, 2, D], f8, isOutput=False)
    if not zb:
        bq_d = nc.declare_dram_parameter("bq", [128, 1], f32, isOutput=False)
        bk_d = nc.declare_dram_parameter("bk", [128, 1], f32, isOutput=False)
        bv_d = nc.declare_dram_parameter("bv", [64, HLOC], f32, isOutput=False)
    if ln_affine:
        gam_d = nc.declare_dram_parameter("gamma", [D], f32, isOutput=False)
        bet_d = nc.declare_dram_parameter("beta", [D], f32, isOutput=False)
    out_d = nc.declare_dram_parameter("out", [RLOC, D], f32, isOutput=True)

    with tile.TileContext(nc) as tc:
        with (
            tc.tile_pool(name="singles", bufs=1) as singles,
            tc.tile_pool(name="temps", bufs=4) as temps,
            tc.tile_pool(name="psA", bufs=2, space="PSUM") as psA,
            tc.tile_pool(name="psB", bufs=2, space="PSUM") as psB,
            tc.tile_pool(name="psP", bufs=2, space="PSUM") as psP,
            tc.tile_pool(name="dram", bufs=1, space="DRAM") as dram,
        ):
            for _rep in range(reps):  # >1 only for benchmarking
                # AllToAll buffers, fp8 normalized ctx^T. Input rows
                # [128*o : 128*(o+1)] hold this core's 128 head-dims for
                # owner-core o's 512 q-rows; output rows [128*r : ...] are
                # core r's head dims (= global dims 128r..) for MY rows.
                a2a_in = dram.tile([NC * 128, RLOC], f8)
                a2a_out = dram.tile([NC * 128, RLOC], f8)

                # ---- small constants first ----
                wq_sb = singles.tile([128, 4, 2, 128], f8)
                wk_sb = singles.tile([128, 4, 2, 128], f8)
                wv_sb = singles.tile([128, 4, 2, 128], f8)
                for w_sb, w_d in ((wq_sb, wq_d), (wk_sb, wk_d), (wv_sb, wv_d)):
                    nc.sync.dma_start(w_sb, w_d[:, :, :, :])
                if not zb:
                    bq_sb = singles.tile([128, 1], f32)
                    nc.sync.dma_start(bq_sb, bq_d[:, :])
                    bk_sb = singles.tile([128, 1], f32)
                    nc.sync.dma_start(bk_sb, bk_d[:, :])
                    bv_sb = singles.tile([64, HLOC], f32)
                    nc.sync.dma_start(bv_sb, bv_d[:, :])
                ones_sb = singles.tile([1, 64], f8)
                nc.vector.memset(ones_sb, 1.0)
                eps_sb = singles.tile([128, 1], f32)
                nc.vector.memset(eps_sb, EPS)
                rsq_dummy = singles.tile([1, 1], f32)

                # ---- x^T fp8 pair-tiles: 8 stripes of 512KB (DMA calls cost
                # ~2.2us each on HW, so few big transfers), batch-0 columns
                # first, split across the SP and ACT hardware DGE queues ----
                xT_sb = [
                    singles.tile([128, 2, ROWS], f8, tag=f"xT{j}", name=f"xT{j}")
                    for j in range(4)
                ]
                for b in range(B):
                    for j in range(4):
                        eng = nc.sync if j % 2 == 0 else nc.scalar
                        eng.dma_start(
                            xT_sb[j][:, :, b * S : (b + 1) * S],
                            xT8_d[:, j, :, b * S : (b + 1) * S],
                        )

                # persistent per-batch projection outputs (separate tiles so
                # batch-1 writes don't false-WAR batch-0 attention reads)
                qT_sb = [
                    singles.tile([128, S], f8, tag=f"qT{b}", name=f"qT{b}")
                    for b in range(B)
                ]
                kT_sb = [
                    singles.tile([128, S], f8, tag=f"kT{b}", name=f"kT{b}")
                    for b in range(B)
                ]
                # v stationary per (head, ktile), padded to 128 columns for
                # DoubleRow ldweights (M must be 64-contiguous or 128):
                # cols 0-63 = v, col 64 = ones (denominator row), 65-127
                # unused (zeroed once; their PSUM rows are never read)
                v_sb = [
                    singles.tile([128, HLOC, KT, 128], f8, tag=f"v{b}", name=f"v{b}")
                    for b in range(B)
                ]
                nc.vector.memset(v_sb[0], 0.0)
                nc.gpsimd.memset(v_sb[1], 0.0)
                for b in range(B):
                    nc.vector.memset(v_sb[b][:, :, :, 64:65], 1.0)

                def qk_copy(dst, src, bias_sb, eng):
                    """PSUM f32 -> SBUF fp8 (+bias when not zb).

                    GPSIMD can't read PSUM, so only ACT ("A") / DVE ("D")."""
                    if zb:
                        if eng == "A":
                            nc.scalar.activation(out=dst, in_=src, func=AF.Copy)
                        else:
                            nc.vector.tensor_copy(dst, src)
                    else:
                        if eng == "A":
                            nc.scalar.activation(
                                out=dst, in_=src, func=AF.Identity, bias=bias_sb
                            )
                        else:
                            nc.vector.tensor_scalar_add(dst, src, bias_sb)

                def qk_piece(b, sc, which, eng):
                    """Generator: one q or k projection chunk, yielding after
                    each DR matmul and after the copy-out."""
                    lo = b * S + sc * 512
                    w_sb = wq_sb if which == "q" else wk_sb
                    dstT = qT_sb[b] if which == "q" else kT_sb[b]
                    bias = None
                    if not zb:
                        bias = bq_sb if which == "q" else bk_sb
                    ps = psP.tile([128, 512], f32, tag="p", name=f"ps{which}{b}{sc}")
                    for j in range(4):
                        nc.tensor.matmul(
                            ps,
                            w_sb[:, j],
                            xT_sb[j][:, :, lo : lo + 512],
                            start=(j == 0),
                            stop=(j == 3),
                            perf_mode=DR,
                        )
                        yield
                    qk_copy(dstT[:, sc * 512 : (sc + 1) * 512], ps, bias, eng)
                    yield

                def v_piece(b, rt, eng):
                    """Generator: one v projection row-tile (x-rows stationary
                    so the PSUM comes out [x-rows, v-dims], matching the
                    attn@V stationary layout with no transpose)."""
                    lo = b * S + rt * 128
                    psv = psP.tile([128, 128], f32, tag="p", name=f"psv{b}{rt}")
                    for j in range(4):
                        nc.tensor.matmul(
                            psv,
                            xT_sb[j][:, :, lo : lo + 128],
                            wv_sb[:, j],
                            start=(j == 0),
                            stop=(j == 3),
                            perf_mode=DR,
                        )
                        if j % 2 == 1:
                            yield
                    dst = v_sb[b][:, :, rt, 0:64]
                    src = psv.rearrange("p (h c) -> p h c", c=64)
                    if eng == "A":
                        nc.scalar.activation(out=dst, in_=src, func=AF.Copy)
                    else:
                        nc.vector.tensor_copy(dst, src)
                    yield

                def chain(*gens):
                    for g in gens:
                        yield from g

                # exp engines: ACT (native Exp) and DVE (schraudolph); GPSIMD
                # can't read the PSUM scores.
                def emit_exp(eng, ex, sg):
                    flat = ex.rearrange("p i n -> p (i n)")
                    if eng == "A":
                        nc.scalar.activation(
                            out=flat, in_=sg, func=AF.Exp, scale=EXP_SCALE
                        )
                    else:
                        nc.vector.tensor_scalar(
                            out=flat.bitcast(i8),
                            in0=sg,
                            scalar1=EXP_SCALE * SCH_A,
                            scalar2=SCH_B,
                            op0=OP.mult,
                            op1=OP.add,
                        )

                # per-head accumulation buffers for the normalized ctx of all
                # four q-chunks of a batch; shipped as ONE DMA per (b, h)
                ctxN_all = [
                    singles.tile([64, QC, 512], f8, tag=f"cNA{h}", name=f"cNA{h}")
                    for h in range(HLOC)
                ]

                def attend_epilogue(b, qc, ctx_ps):
                    """normalize both heads' ctx^T by their denominator rows
                    (bit-trick reciprocal), cast fp8, stage into ctxN_all.
                    DVE ops may read at most one PSUM operand, so the
                    broadcast reciprocal plane goes PSUM -> SBUF once."""
                    rep = psP.tile([128, 512], f32, tag="p", name="rep")
                    for h in range(HLOC):
                        rec = temps.tile([1, 512], f32, tag="rec", name=f"rec{h}")
                        nc.vector.tensor_scalar(
                            out=rec.bitcast(i32),
                            in0=ctx_ps[h][64:65, :].bitcast(i32),
                            scalar1=-1,
                            scalar2=RCP_K,
                            op0=OP.mult,
                            op1=OP.add,
                        )
                        rec8 = temps.tile([1, 512], f8, tag="rec8", name=f"rec8{h}")
                        nc.gpsimd.tensor_scalar_mul(rec8, rec, float(SR))
                        nc.tensor.matmul(
                            rep[h * 64 : (h + 1) * 64, :],
                            ones_sb,
                            rec8,
                            start=True,
                            stop=True,
                            tile_position=(0, h * 64),
                        )
                    repS = temps.tile([128, 512], f32, tag="repS")
                    nc.vector.tensor_copy(repS, rep)
                    for h in range(HLOC):
                        hp = h * 64
                        ctxN = ctxN_all[h][:, qc, :]
                        if zb:
                            nc.vector.tensor_mul(
                                ctxN, ctx_ps[h][0:64, :], repS[hp : hp + 64, :]
                            )
                        else:
                            ctxE = temps.tile([64, 512], f32, tag="ctxE")
                            nc.vector.tensor_mul(
                                ctxE, ctx_ps[h][0:64, :], repS[hp : hp + 64, :]
                            )
                            # bv_sb arrives host-scaled by SV*SR
                            nc.vector.tensor_scalar_add(
                                ctxN, ctxE, bv_sb[:, h : h + 1]
                            )

                def ship_batch(b):
                    """one strided DMA per head: [64, QC, 512] -> a2a_in rows
                    {(b*QC+qc)*128 + h*64 + d}."""
                    a2a_v = a2a_in.rearrange("(o p) n -> p o n", p=128)
                    for h in range(HLOC):
                        hp = h * 64
                        nc.sync.dma_start(
                            a2a_v[hp : hp + 64, b * QC : (b + 1) * QC, :],
                            ctxN_all[h],
                        )

                ng = KT // 4 if ablate == "half_attend" else KT // 2

                def attend_batch(b, weave=None, aux=(), dve_mod=(2, 3), rate=1):
                    """scores (fp8, K=64) -> exp (ACT/DVE, fp8 out) -> ctx^T
                    via DoubleRow over k-tile pairs (+denom row), with depth-2
                    deferral of the ctx matmuls and the normalize epilogue
                    emitted in the window between chunks.

                    weave: generator whose next() emits one foreign PE piece;
                    `rate` pieces are consumed per unit.
                    aux: callables emitted one per unit from unit 0 (used to
                    flush the previous batch's deferred tail).
                    dve_mod: (2g+h) % 8 values routed to DVE exp.
                    Returns (pend, last_epi) for the caller to flush.
                    """
                    aux = deque(aux)
                    pend = deque()  # (ctx_ps, h, ex, g)
                    last_epi = None
                    done = object()  # generators yield None; sentinel differs

                    def flush_one():
                        ctx_ps_, h_, ex_, g_ = pend.popleft()
                        nc.tensor.matmul(
                            ctx_ps_[h_],
                            v_sb[b][:, h_, 2 * g_ : 2 * g_ + 2, :],
                            ex_[:, :, :],
                            start=(g_ == 0),
                            stop=(g_ == ng - 1),
                            perf_mode=DR,
                        )

                    for qc in range(QC):
                        qlo = qc * 512
                        ctx_ps = [
                            psB.tile([128, 512], f32, tag="b", name=f"ctx{h}")
                            for h in range(HLOC)
                        ]
                        for g in range(ng):
                            for h in range(HLOC):
                                hp = h * 64
                                sg = psA.tile(
                                    [128, 1024], f32, tag="a", name=f"sg{h}"
                                )
                                for jj in range(2):
                                    klo = (2 * g + jj) * 128
                                    nc.tensor.matmul(
                                        sg[:, jj * 512 : (jj + 1) * 512],
                                        kT_sb[b][hp : hp + 64, klo : klo + 128],
                                        qT_sb[b][hp : hp + 64, qlo : qlo + 512],
                                        start=True,
                                        stop=True,
                                    )
                                ex = temps.tile(
                                    [128, 2, 512], f8, tag="exps", name=f"ex{h}"
                                )
                                eng = "D" if (2 * g + h) % 8 in dve_mod else "A"
                                emit_exp(eng, ex, sg)
                                if aux:
                                    aux.popleft()()
                                for _ in range(rate):
                                    if weave is None:
                                        break
                                    if next(weave, done) is done:
                                        weave = None
                                pend.append((ctx_ps, h, ex, g))
                                if len(pend) > 2:
                                    flush_one()
                                # window between chunks: the previous chunk's
                                # last ctx was just flushed above; normalize
                                # it before the next flush (the new chunk's
                                # first ctx) reuses its PSUM slot
                                if g == 0 and h == 1 and last_epi is not None:
                                    attend_epilogue(*last_epi)
                                    last_epi = None
                        last_epi = (b, qc, ctx_ps)
                    return pend, last_epi

                def flush_attend(b, pend, last_epi):
                    while pend:
                        ctx_ps_, h_, ex_, g_ = pend.popleft()
                        nc.tensor.matmul(
                            ctx_ps_[h_],
                            v_sb[b][:, h_, 2 * g_ : 2 * g_ + 2, :],
                            ex_[:, :, :],
                            start=(g_ == 0),
                            stop=(g_ == ng - 1),
                            perf_mode=DR,
                        )
                    if last_epi is not None:
                        attend_epilogue(*last_epi)

                # ---- batch-0 projection head: k first, then q chunk 0, then
                # all of v (batch 0's first q-chunk consumes every v k-tile
                # within its first 16 units, so v0 can't be woven); q chunks
                # 1-3 and all of batch 1's projections weave into the attend
                # streams as PE-slack filler (weave mode) or run as plain
                # blocks (no-weave mode) ----
                for sc in range(QC):
                    for _ in qk_piece(0, sc, "k", "A"):
                        pass
                for _ in qk_piece(0, 0, "q", "A"):
                    pass
                for rt in range(KT):
                    for _ in v_piece(0, rt, "A"):
                        pass

                if WEAVE:
                    # 64 yields, one per batch-0 attend unit; ordered so each
                    # piece lands before its first reader (q0 chunk sc is
                    # read from unit 16*sc; batch-1 pieces in attend 1)
                    weave0 = chain(
                        qk_piece(0, 1, "q", "D"),
                        qk_piece(0, 2, "q", "D"),
                        qk_piece(0, 3, "q", "D"),
                        *[qk_piece(1, sc, "k", "D") for sc in range(QC)],
                        qk_piece(1, 0, "q", "D"),
                        *[v_piece(1, rt, "D") for rt in range(0, 8)],
                    )
                else:
                    for sc in range(1, QC):
                        for _ in qk_piece(0, sc, "q", "A"):
                            pass
                    weave0 = None
                pend0, epi0 = attend_batch(0, weave=weave0, dve_mod=(2, 3))

                # batch-1 leftovers + phase-4 constant loads weave into the
                # batch-1 attend stream (their deadlines are chunks 1-3)
                def load_tail_consts():
                    wo_sb_l = singles.tile([128, 4, 2, D], f8, name="wo_sb")
                    nc.sync.dma_start(wo_sb_l, wo_d[:, :, :, :])
                    tail_tiles["wo"] = wo_sb_l
                    if ln_affine:
                        gam_sb = singles.tile([128, D], f32, name="gam_sb")
                        gap = gam_d.ap()
                        nc.sync.dma_start(
                            gam_sb,
                            bass.AP(
                                tensor=gap.tensor,
                                offset=gap.offset,
                                ap=[[0, 128], gap.ap[0]],
                            ),
                        )
                        tail_tiles["gam"] = gam_sb
                        bet_sb = singles.tile([128, D], f32, name="bet_sb")
                        bap = bet_d.ap()
                        nc.sync.dma_start(
                            bet_sb,
                            bass.AP(
                                tensor=bap.tensor,
                                offset=bap.offset,
                                ap=[[0, 128], bap.ap[0]],
                            ),
                        )
                        tail_tiles["bet"] = bet_sb
                    xb_sb_l = singles.tile([128, RLOC // 128, D], f32, name="xb_sb")
                    nc.sync.dma_start(
                        xb_sb_l, xb_d.ap().rearrange("(t p) d -> p t d", p=128)
                    )
                    tail_tiles["xb"] = xb_sb_l

                tail_tiles = {}

                def flush0_ctx():
                    # flush batch 0's two deferred ctx matmuls
                    while pend0:
                        ctx_ps_, h_, ex_, g_ = pend0.popleft()
                        nc.tensor.matmul(
                            ctx_ps_[h_],
                            v_sb[0][:, h_, 2 * g_ : 2 * g_ + 2, :],
                            ex_[:, :, :],
                            start=(g_ == 0),
                            stop=(g_ == ng - 1),
                            perf_mode=DR,
                        )

                def epi0_fn():
                    attend_epilogue(*epi0)

                def ship0_fn():
                    ship_batch(0)
                    load_tail_consts()

                if WEAVE:
                    # batch-1 leftovers: v k-tiles 8-15 are read from unit 10
                    # on (ctx for group g lands at unit 2g+3), so consume 2
                    # pieces per unit until the generator drains (~unit 19)
                    weave1 = chain(
                        *[v_piece(1, rt, "D") for rt in range(8, 12)],
                        qk_piece(1, 1, "q", "D"),
                        *[v_piece(1, rt, "D") for rt in range(12, KT)],
                        qk_piece(1, 2, "q", "D"),
                        qk_piece(1, 3, "q", "D"),
                    )
                    aux1 = (flush0_ctx, epi0_fn, ship0_fn)
                else:
                    flush0_ctx()
                    epi0_fn()
                    ship0_fn()
                    for sc in range(QC):
                        for _ in qk_piece(1, sc, "k", "D"):
                            pass
                        for _ in qk_piece(1, sc, "q", "D"):
                            pass
                    for rt in range(KT):
                        for _ in v_piece(1, rt, "D"):
                            pass
                    weave1, aux1 = None, ()

                pend1, epi1 = attend_batch(
                    1,
                    weave=weave1,
                    aux=aux1,
                    dve_mod=(2, 3, 4),
                    rate=2,
                )
                flush_attend(1, pend1, epi1)
                # preload the Sqrt table set while the AllToAll runs (ACT is
                # idle; keeps the exp->sqrt switch off the critical path)
                nc.scalar.activation(
                    out=rsq_dummy, in_=eps_sb[0:1, :], func=AF.Sqrt
                )
                ship_batch(1)

                # ---- AllToAll: head-sharded ctx^T -> row-sharded ctx^T ----
                if ablate == "no_a2a":
                    a2a_out = a2a_in
                elif sim1 or ablate == "local_a2a":
                    for o in range(NC):
                        nc.sync.dma_start(
                            a2a_out[o * 128 : (o + 1) * 128, :],
                            a2a_in[o * 128 : (o + 1) * 128, :],
                        )
                else:
                    nc.gpsimd.collective_compute(
                        "AllToAll",
                        OP.bypass,
                        replica_groups=[list(range(NC))],
                        ins=[a2a_in.opt()],
                        outs=[a2a_out.opt()],
                    )

                # ---- out-projection (DoubleRow) + residual + LayerNorm,
                # software-pipelined: pso is one [128,1024] tile per row-tile
                # (2 in flight), so the next row-tile's matmuls never wait on
                # this one's DVE epilogue ----
                wo_sb = tail_tiles["wo"]
                xb_sb = tail_tiles["xb"]
                ct_sb = [
                    singles.tile([128, 2, RLOC], f8, tag=f"ct{j}", name=f"ct{j}")
                    for j in range(4)
                ]
                a2a_ov = a2a_out.rearrange("(j i p) n -> p j i n", i=2, p=128)
                for j in range(4):
                    eng = nc.sync if j % 2 == 0 else nc.scalar
                    eng.dma_start(ct_sb[j], a2a_ov[:, j, :, :])

                def tail_mms(t):
                    pso = psA.tile([128, D], f32, tag="a", name=f"pso{t}")
                    for eh in range(2):
                        for j in range(4):
                            nc.tensor.matmul(
                                pso[:, eh * 512 : (eh + 1) * 512],
                                ct_sb[j][:, :, t * 128 : (t + 1) * 128],
                                wo_sb[:, j, :, eh * 512 : (eh + 1) * 512],
                                start=(j == 0),
                                stop=(j == 3),
                                perf_mode=DR,
                            )
                    return pso

                def tail_epilogue(t, pso):
                    y_sb = temps.tile([128, D], f32, tag="y")
                    # residual with fp8 descale: y = out*2^-18 + (x + bo)
                    # (scalar_tensor_tensor is DVE-only: walrus rejects it on
                    # the Pool engine)
                    for eh in range(2):
                        nc.vector.scalar_tensor_tensor(
                            out=y_sb[:, eh * 512 : (eh + 1) * 512],
                            in0=pso[:, eh * 512 : (eh + 1) * 512],
                            scalar=OUT_DESCALE,
                            in1=xb_sb[:, t, eh * 512 : (eh + 1) * 512],
                            op0=OP.mult,
                            op1=OP.add,
                        )
                    # LayerNorm over D=1024 (free dim)
                    stats = temps.tile([128, 2, 6], f32, tag="stats")
                    for i in range(2):
                        nc.vector.bn_stats(
                            out=stats[:, i, :],
                            in_=y_sb[:, i * 512 : (i + 1) * 512],
                        )
                    mv = temps.tile([128, 2], f32, tag="mv")
                    nc.vector.bn_aggr(out=mv, in_=stats)
                    # rstd = 1/Sqrt(var + eps); table preloaded behind the a2a
                    nc.scalar.activation(
                        out=mv[:, 1:2], in_=mv[:, 1:2], func=AF.Sqrt, bias=eps_sb
                    )
                    nc.vector.reciprocal(mv[:, 1:2], mv[:, 1:2])
                    # y_norm = (y - mean) * rstd on GPSIMD (SBUF-only engine)
                    yo = temps.tile([128, D], f32, tag="yo")
                    nc.gpsimd.tensor_scalar(
                        out=yo,
                        in0=y_sb,
                        scalar1=mv[:, 0:1],
                        scalar2=mv[:, 1:2],
                        op0=OP.subtract,
                        op1=OP.mult,
                    )
                    if ln_affine:
                        nc.vector.tensor_mul(yo, yo, tail_tiles["gam"])
                        nc.gpsimd.tensor_add(yo, yo, tail_tiles["bet"])
                    nc.sync.dma_start(out_d[t * 128 : (t + 1) * 128, :], yo)

                nt = 0 if ablate == "no_tail" else RLOC // 128
                prev = None
                for t in range(nt):
                    pso = tail_mms(t)
                    if prev is not None:
                        tail_epilogue(*prev)
                    prev = (t, pso)
                if prev is not None:
                    tail_epilogue(*prev)

    nc.compile()
    return nc


def _numpy_reference(x, mask, Wq, bq, Wk, bk, Wv, bv, Wo, bo, gamma, beta):
    """Fallback for a non-all-ones mask (can't occur per the input spec)."""
    b = x.shape[0]
    x64 = x.astype(np.float64)

    def split(t):
        return t.reshape(b, -1, H, HD).transpose(0, 2, 1, 3)

    q = split(x64 @ Wq + bq)
    k = split(x64 @ Wk + bk)
    v = split(x64 @ Wv + bv)
    scores = np.einsum("bhqd,bhkd->bhqk", q, k) * NORM
    scores = np.where(mask == 0, -1e9, scores)
    scores -= scores.max(axis=-1, keepdims=True)
    e = np.exp(scores)
    attn = e / e.sum(axis=-1, keepdims=True)
    ctx = np.einsum("bhqk,bhkd->bhqd", attn, v)
    ctx = ctx.transpose(0, 2, 1, 3).reshape(b, -1, D)
    out = ctx @ Wo + bo
    y = out + x64
    mu = y.mean(-1, keepdims=True)
    var = y.var(-1, keepdims=True)
    return ((y - mu) / np.sqrt(var + EPS) * gamma + beta).astype(np.float32)


def kernel(x, mask, Wq, bq, Wk, bk, Wv, bv, Wo, bo, gamma, beta):
    x = np.asarray(x, dtype=np.float32)
    mask = np.asarray(mask)
    Wq, bq = np.asarray(Wq, np.float32), np.asarray(bq, np.float32)
    Wk, bk = np.asarray(Wk, np.float32), np.asarray(bk, np.float32)
    Wv, bv = np.asarray(Wv, np.float32), np.asarray(bv, np.float32)
    Wo, bo = np.asarray(Wo, np.float32), np.asarray(bo, np.float32)
    gamma, beta = np.asarray(gamma, np.float32), np.asarray(beta, np.float32)

    if not np.all(mask):
        return _numpy_reference(x, mask, Wq, bq, Wk, bk, Wv, bv, Wo, bo, gamma, beta)

    ln_affine = not (np.all(gamma == 1.0) and np.all(beta == 0.0))
    zb = bool(
        np.all(bq == 0.0) and np.all(bk == 0.0) and np.all(bv == 0.0)
    )
    key = ("nc", ln_affine, zb)
    if key not in _CACHE:
        _CACHE[key] = _build(ln_affine=ln_affine, zb=zb)
    nc = _CACHE[key]

    e4 = ml_dtypes.float8_e4m3

    def to_pair(w, scale):  # [1024, M] -> [128, 4, 2, M], contraction 128*(2j+i)+p
        m = w.shape[1]
        return np.ascontiguousarray(
            (w * scale).reshape(4, 2, 128, m).transpose(2, 0, 1, 3)
        ).astype(e4)

    x2 = x.reshape(ROWS, D)
    xT8 = np.ascontiguousarray(
        x2.T.reshape(4, 2, 128, ROWS).transpose(2, 0, 1, 3)
    ).astype(e4)
    wo_8 = to_pair(Wo, SW)
    in_maps = []
    for c in range(NC):
        hc = c * HLOC  # first head on this core
        d0 = hc * HD  # its first column/row in the D dim
        m = {
            "xT8": xT8,
            "xb": np.ascontiguousarray(x2[c * RLOC : (c + 1) * RLOC]) + bo,
            "wq": to_pair(Wq[:, d0 : d0 + 128], SQ),
            "wk": to_pair(Wk[:, d0 : d0 + 128], SK),
            "wv": to_pair(Wv[:, d0 : d0 + 128], SV),
            "wo": wo_8,
        }
        if not zb:
            m["bq"] = (np.ascontiguousarray(bq[d0 : d0 + 128]) * SQ).reshape(128, 1)
            m["bk"] = (np.ascontiguousarray(bk[d0 : d0 + 128]) * SK).reshape(128, 1)
            m["bv"] = np.ascontiguousarray(
                bv[d0 : d0 + 128].reshape(HLOC, HD).T * (SV * SR)
            )
        if ln_affine:
            m["gamma"] = gamma
            m["beta"] = beta
        in_maps.append(m)

    res = run_bass_kernel_spmd(nc, in_maps, list(range(NC)))
    out = np.concatenate([res.results[c]["out"] for c in range(NC)], axis=0)
    return out.reshape(B, S, D).astype(np.float32)
